# revision 1
# baseline (speedup 1.0000x reference)
"""Trainium2 Bass kernel for nn_PointEncoder (B=16, N=8192, L=512, D=384).

Sharding: data-parallel over batch, 2 batches per NeuronCore x 8 cores,
no collectives; full inputs sharded / outputs gathered on host.

Per core, per batch (all on-chip, streaming N in 512-point chunks with a
software-pipelined 1-chunk skew; batch b-1's epilogue overlaps batch b):
  stage A: point-MLP 3->D->D->D->D in transposed [D, chunk] layout
           (fp32r matmuls = full PE speed), LN stats via ones-matmul over
           partitions, rsqrt via ln/exp (one ACT table set for the whole
           stream), mean/scale broadcast via K=1 PE matmuls, normalize.
  stage B: V = cn @ wv (bf16); attention scores simT = cn^T @ (wk@q^T)
           -- wk is folded into the query side on the host, and the
           k-bias drops exactly (cancels in softmax); exp with no max
           subtraction (logits provably bounded); attn@V accumulated in
           PSUM across all 16 chunks; denominator accumulated on DVE.
  epilogue (split in 3, overlapped with the next batch's chunks):
           normalize + v-bias, output projection in both layouts (no
           transposes anywhere), PreNorm GEGLU FF (bf16), residuals.

Host prep inside kernel(): pre-transpose x, fold LN gains/biases into
adjacent weights, precompute the batch-independent query path, bf16-cast
FF weights, build wq2 = wk_folded @ q^T.
"""

import math
import numpy as np
import ml_dtypes

import concourse.bass as bass
import concourse.tile as tile
import concourse.mybir as mybir
from concourse import bacc

P = 128
B, N_FULL, L, D = 16, 8192, 512, 384
FF = 4 * D  # 1536
FF2 = 2 * FF  # 3072
DT = D // P  # 3
LT = L // P  # 4
FFT = FF // P  # 12
CHUNK = 512
CT = CHUNK // P  # 4
NCORES = 8
BPC = B // NCORES  # 2

f32 = mybir.dt.float32
f32r = mybir.dt.float32r
bf16 = mybir.dt.bfloat16
AF = mybir.ActivationFunctionType
ALU = mybir.AluOpType

EPS = 1e-5
SCALE = 1.0 / math.sqrt(D)

# Steer the activation-table-load chooser to 'natural_log_exp_and_others'
# (which contains BOTH ln and exp) instead of thrashing between
# 'natural_log' and 'exp_and_others' every chunk.  We only hide ln/exp
# from the sets we don't want chosen; emitted set ids stay truthful.
_tables_patched = False


def _patch_act_tables():
    global _tables_patched
    if _tables_patched:
        return
    from concourse import hw_specs, bacc as _bacc
    orig = hw_specs.get_activation_tables

    def patched(arch):
        t = dict(orig(arch))
        if "natural_log_exp_and_others" in t:
            if "exp_and_others" in t:
                t["exp_and_others"] = t["exp_and_others"] - {AF.Exp}
            if "natural_log" in t:
                t["natural_log"] = t["natural_log"] - {AF.Ln}
        return t

    _bacc.get_activation_tables = patched
    _tables_patched = True


def _bcast_ap(ap, p=P):
    """DRAM AP [n] -> [p, n] with partition step 0 (replicated load)."""
    return bass.AP(tensor=ap.tensor, offset=ap.offset, ap=[[0, p], *ap.ap])


def build_nc(n_points=N_FULL, bpc=BPC, gelu_af=None, repeat=None):
    nchunks = n_points // CHUNK
    if gelu_af is None:
        gelu_af = AF.Gelu
    _patch_act_tables()
    nc = bacc.Bacc("TRN2", target_bir_lowering=False, debug=False,
                   enable_asserts=False)

    din = {}
    def di(name, shape, dtype=f32):
        din[name] = nc.dram_tensor(name, list(shape), dtype,
                                   kind="ExternalInput").ap()
        return din[name]

    xT = di("xT", [bpc, 3, n_points], f32r)
    wq2 = di("wq2", [D, L], f32r)
    lqT = di("lqT", [D, L])
    lqn = di("lqn", [L, D])
    w0 = di("w0", [3, D], f32r)
    w1 = di("w1", [D, D], f32r)
    w2 = di("w2", [D, D], f32r)
    w3 = di("w3", [D, D], f32r)
    b0 = di("b0", [D])
    b1 = di("b1", [D])
    b2 = di("b2", [D])
    b3 = di("b3", [D])
    wkv = di("wkv", [D, 2 * D], f32r)   # ln_ctx_g pre-folded
    bkv = di("bkv", [2 * D])      # ln_ctx_b @ wkv
    wo = di("wo", [D, D], f32r)
    bo = di("bo", [D])
    fw1 = di("fw1", [D, FF2], bf16)  # ln_ff_g pre-folded, bf16
    bu = di("bu", [FF2])             # ff_b1 + ln_ff_b @ ff_w1
    fw2 = di("fw2", [FF, D], bf16)
    fb2 = di("fb2", [D])
    y = nc.dram_tensor("y", [bpc, L, D], f32, kind="ExternalOutput").ap()

    with tile.TileContext(nc) as tc:
        with tc.tile_pool(name="singles", bufs=1) as singles, \
             tc.tile_pool(name="work", bufs=1) as work, \
             tc.tile_pool(name="psum", bufs=1, space="PSUM") as psum:

            # ---------------- load params ----------------
            def ld(name, ap, shape, dtype=f32, src=None, eng=None):
                t = singles.tile(shape, dtype, name=name)
                (eng or nc.sync).dma_start(t, src if src is not None else ap)
                return t

            r3 = lambda a: a.rearrange("(t p) m -> p t m", p=P)
            rc = lambda a: a.rearrange("(t p) -> p t", p=P)

            # prefetch the very first x chunk so the PE can start immediately
            xT_pre = work.tile([3, CHUNK], f32r, tag="xT", bufs=2,
                               name="xT_pre")
            nc.sync.dma_start(xT_pre, xT[0, :, 0:CHUNK])
            # critical path first (chunk-stream weights), on the sync queue
            w0_sb = ld("w0_sb", w0, [3, D], f32r)
            b0_sb = ld("b0_sb", None, [P, DT], src=rc(b0))
            w1_sb = ld("w1_sb", None, [P, DT, D], f32r, src=r3(w1))
            b1_sb = ld("b1_sb", None, [P, DT], src=rc(b1))
            w2_sb = ld("w2_sb", None, [P, DT, D], f32r, src=r3(w2))
            b2_sb = ld("b2_sb", None, [P, DT], src=rc(b2))
            w3_sb = ld("w3_sb", None, [P, DT, D], f32r, src=r3(w3))
            b3_sb = ld("b3_sb", None, [P, DT], src=rc(b3))
            wkv_sb = ld("wkv_sb", None, [P, DT, 2 * D], f32r, src=r3(wkv))
            wq2_sb = ld("wq2_sb", None, [P, DT, L], f32r, src=r3(wq2))
            bv_c = ld("bv_c", None, [P, DT], src=rc(bkv[D:2 * D]))
            # epilogue-only params on the gpsimd queue (off the critical path)
            g = nc.gpsimd
            wo_sb = ld("wo_sb", None, [P, DT, D], f32r, src=r3(wo), eng=g)
            fw1_sb = ld("fw1_sb", None, [P, DT, FF2], bf16, src=r3(fw1), eng=g)
            fw2_sb = ld("fw2_sb", None, [P, FFT, D], bf16, src=r3(fw2), eng=g)
            lqT_sb = ld("lqT_sb", None, [P, DT, L], src=r3(lqT), eng=g)
            lqn_sb = ld("lqn_sb", None, [P, LT, D],
                        src=lqn.rearrange("(t p) d -> p t d", p=P), eng=g)
            bo_sb = ld("bo_sb", None, [P, DT], src=rc(bo), eng=g)
            bu_sb = ld("bu_sb", None, [P, 2 * FFT], src=rc(bu), eng=g)
            bo_b = ld("bo_b", None, [P, D], src=_bcast_ap(bo), eng=g)
            fb2_b = ld("fb2_b", None, [P, D], src=_bcast_ap(fb2), eng=g)

            ones_tmp = singles.tile([P, 1], f32)
            nc.vector.memset(ones_tmp, 1.0 / D)
            ones_over_D = singles.tile([P, 1], f32r)
            nc.vector.tensor_copy(ones_over_D, ones_tmp)
            ones_tmp2 = singles.tile([P, 1], f32)
            nc.vector.memset(ones_tmp2, 1.0)
            ones_col = singles.tile([P, 1], f32r)
            nc.vector.tensor_copy(ones_col, ones_tmp2)
            ones_tmp3 = singles.tile([1, P], f32)
            nc.vector.memset(ones_tmp3, 1.0)
            ones_row = singles.tile([1, P], f32r)
            nc.vector.tensor_copy(ones_row, ones_tmp3)
            eps_col = singles.tile([P, 1], f32)
            nc.vector.memset(eps_col, EPS)

            mlp_w = [(w0_sb, b0_sb), (w1_sb, b1_sb), (w2_sb, b2_sb),
                     (w3_sb, b3_sb)]

            def _run_batches():
                def make_batch(b):
                    # attention accumulators, held across the whole chunk loop
                    acc_ps = psum.tile([P, DT, L], f32, tag="acc", name=f"acc{b}")
                    den_sb = work.tile([P, L], f32r, tag="den_acc", bufs=1,
                                       name=f"den_sb{b}")

                    # LN helper: given hT tiles [P, DT, CH] produce
                    # (m_b PSUM [P,CH], a_sb SBUF [P,CH]) per-column stats.
                    def ln_stats(h_sb, ch, uid):
                        ps_m = psum.tile([1, ch], f32, tag="stat", bufs=2,
                                         name=f"psm{uid}")
                        ps_sq = psum.tile([1, ch], f32, tag="stat", bufs=2,
                                          name=f"pssq{uid}")
                        for kt in range(DT):
                            sq_sb = work.tile([P, ch], f32r, tag="sq", bufs=3,
                                              name=f"sq{uid}_{kt}")
                            nc.scalar.activation(sq_sb, h_sb[:, kt, :], AF.Square)
                            nc.tensor.matmul(ps_m, ones_over_D,
                                             h_sb[:, kt, :],
                                             start=(kt == 0), stop=(kt == DT - 1))
                            nc.tensor.matmul(ps_sq, ones_over_D,
                                             sq_sb,
                                             start=(kt == 0), stop=(kt == DT - 1))
                        # row-level stats math on [1, ch], then PE broadcasts
                        m_row = work.tile([1, ch], f32r, tag="row", bufs=2,
                                          name=f"mrow{uid}")
                        nc.vector.tensor_copy(m_row, ps_m)
                        msq_row = work.tile([1, ch], f32, tag="row", bufs=2,
                                            name=f"msqrow{uid}")
                        nc.vector.tensor_tensor(msq_row, m_row, m_row, ALU.mult)
                        var_row = work.tile([1, ch], f32, tag="row", bufs=2,
                                            name=f"varrow{uid}")
                        nc.vector.tensor_tensor(var_row, ps_sq, msq_row,
                                                ALU.subtract)
                        lnv_row = work.tile([1, ch], f32, tag="row", bufs=2,
                                            name=f"lnvrow{uid}")
                        nc.scalar.activation(lnv_row, var_row, AF.Ln,
                                             bias=eps_col[0:1], scale=1.0)
                        a_row = work.tile([1, ch], f32r, tag="row", bufs=2,
                                          name=f"arow{uid}")
                        nc.scalar.activation(a_row, lnv_row, AF.Exp, scale=-0.5)
                        ps_mb = psum.tile([P, ch], f32, tag="stat", bufs=2,
                                          name=f"psmb{uid}")
                        nc.tensor.matmul(ps_mb, ones_row, m_row,
                                         start=True, stop=True)
                        ps_ab = psum.tile([P, ch], f32, tag="stat", bufs=2,
                                          name=f"psab{uid}")
                        nc.tensor.matmul(ps_ab, ones_row, a_row,
                                         start=True, stop=True)
                        return ps_mb, ps_ab

                    def stage_a(c):
                        """MLP + LN stats + normalized cn for chunk c."""
                        uid = f"{b}_{c}"
                        # ---- point MLP, transposed layout ----
                        if b == 0 and c == 0:
                            xT_c = xT_pre
                        else:
                            xT_c = work.tile([3, CHUNK], f32r, tag="xT", bufs=2,
                                             name=f"xT{uid}")
                            nc.sync.dma_start(
                                xT_c, xT[b, :, c * CHUNK:(c + 1) * CHUNK])
                        h_prev = None
                        for li, (w_sb, bcol) in enumerate(mlp_w):
                            h_sb = work.tile([P, DT, CHUNK], f32r, tag="h",
                                             bufs=4, name=f"h{li}_{uid}")
                            for mt in range(DT):
                                ps = psum.tile([P, CHUNK], f32, tag="work", bufs=3,
                                               name=f"psh{li}{mt}_{uid}")
                                if li == 0:
                                    nc.tensor.matmul(
                                        ps, w0_sb[:, mt * P:(mt + 1) * P],
                                        xT_c, start=True, stop=True)
                                else:
                                    for kt in range(DT):
                                        nc.tensor.matmul(
                                            ps,
                                            w_sb[:, kt, mt * P:(mt + 1) * P],
                                            h_prev[:, kt, :],
                                            start=(kt == 0), stop=(kt == DT - 1))
                                if li < 3:
                                    # relu(x + b): h0/h1 on ACT, h2 on DVE
                                    if li < 2:
                                        nc.scalar.activation(
                                            h_sb[:, mt, :], ps, AF.Relu,
                                            bias=bcol[:, mt:mt + 1], scale=1.0)
                                    else:
                                        nc.vector.tensor_scalar(
                                            out=h_sb[:, mt, :], in0=ps,
                                            scalar1=bcol[:, mt:mt + 1], scalar2=0.0,
                                            op0=ALU.add, op1=ALU.max)
                                else:
                                    nc.vector.tensor_scalar(
                                        out=h_sb[:, mt, :], in0=ps,
                                        scalar1=bcol[:, mt:mt + 1], scalar2=None,
                                        op0=ALU.add)
                            h_prev = h_sb

                        # ---- LayerNorm over D (per point) ----
                        ps_mb, ps_ab = ln_stats(h_prev, CHUNK, uid)
                        cn_sb = work.tile([P, DT, CHUNK], f32r, tag="cn", bufs=2,
                                          name=f"cn{uid}")
                        for kt in range(DT):
                            nc.vector.tensor_tensor(cn_sb[:, kt, :],
                                                    h_prev[:, kt, :], ps_mb,
                                                    ALU.subtract)
                            nc.vector.tensor_tensor(cn_sb[:, kt, :],
                                                    cn_sb[:, kt, :], ps_ab, ALU.mult)
                        return cn_sb

                    def stage_b(c, cn_sb):
                        """V, attention scores, exp, attn@V accumulate."""
                        uid = f"{b}_{c}"
                        # ---- V [CHUNK, D] (bias applied in epilogue) ----
                        v_ext = work.tile([P, CT, D], bf16, tag="v", bufs=2,
                                          name=f"v{uid}")
                        for jt in range(CT):
                            ps = psum.tile([P, CHUNK], f32, tag="work", bufs=3,
                                           name=f"psv{jt}_{uid}")
                            for kt in range(DT):
                                nc.tensor.matmul(
                                    ps[:, 0:D],
                                    cn_sb[:, kt, jt * P:(jt + 1) * P],
                                    wkv_sb[:, kt, D:2 * D],
                                    start=(kt == 0), stop=(kt == DT - 1))
                            nc.scalar.activation(v_ext[:, jt, :], ps[:, 0:D],
                                                 AF.Identity, bias=0.0, scale=1.0)

                        # ---- simT = K^T.T @ qT, exp, accumulate attn@V ----
                        expT = work.tile([P, CT, L], bf16, tag="e", bufs=2,
                                         name=f"e{uid}")
                        for jt in range(CT):
                            ps = psum.tile([P, L], f32, tag="work", bufs=3,
                                           name=f"pss{jt}_{uid}")
                            for kt in range(DT):
                                nc.tensor.matmul(
                                    ps, cn_sb[:, kt, jt * P:(jt + 1) * P],
                                    wq2_sb[:, kt, :],
                                    start=(kt == 0), stop=(kt == DT - 1))
                            nc.scalar.activation(expT[:, jt, :], ps, AF.Exp,
                                                 scale=SCALE)
                        first = (c == 0)
                        last = (c == nchunks - 1)
                        for jt in range(CT):
                            for mt in range(DT):
                                nc.tensor.matmul(
                                    acc_ps[:, mt, :],
                                    v_ext[:, jt, mt * P:(mt + 1) * P],
                                    expT[:, jt, :],
                                    start=(first and jt == 0),
                                    stop=(last and jt == CT - 1),
                                    skip_group_check=True)
                            if first and jt == 0:
                                nc.vector.tensor_copy(den_sb, expT[:, jt, :])
                            else:
                                nc.vector.tensor_tensor(den_sb, den_sb,
                                                        expT[:, jt, :], ALU.add)

                    # epilogue part 1: normalize attention output (frees acc)
                    def epi1():
                        ub = f"b{b}"
                        ps_den = psum.tile([1, L], f32, tag="stat", bufs=2,
                                           name=f"psden{ub}")
                        nc.tensor.matmul(ps_den, ones_col, den_sb,
                                         start=True, stop=True)
                        den_row = work.tile([1, L], f32, tag="row", bufs=2,
                                            name=f"den_row{ub}")
                        nc.vector.tensor_copy(den_row, ps_den)
                        rec_f = work.tile([1, L], f32, tag="row", bufs=2,
                                          name=f"rec_f{ub}")
                        nc.vector.reciprocal(rec_f, den_row)
                        rec_row = work.tile([1, L], f32r, tag="row", bufs=2,
                                            name=f"rec_row{ub}")
                        nc.vector.tensor_copy(rec_row, rec_f)
                        ps_rb = psum.tile([P, L], f32, tag="stat", bufs=2,
                                          name=f"psrb{ub}")
                        nc.tensor.matmul(ps_rb, ones_row, rec_row,
                                         start=True, stop=True)
                        rb_sb = work.tile([P, L], f32, tag="sc", bufs=3,
                                          name=f"rb{ub}")
                        nc.vector.tensor_copy(rb_sb, ps_rb)
                        outn = work.tile([P, DT, L], f32r, tag="outn", bufs=1,
                                         name=f"outn{ub}")
                        for dt_ in range(DT):
                            nc.vector.tensor_tensor(outn[:, dt_, :], acc_ps[:, dt_, :],
                                                    rb_sb, ALU.mult)
                            nc.vector.tensor_scalar(
                                out=outn[:, dt_, :], in0=outn[:, dt_, :],
                                scalar1=bv_c[:, dt_:dt_ + 1], scalar2=None,
                                op0=ALU.add)

                        return outn

                    # epilogue part 2: projections + LN_ff chain
                    def epi2(outn):
                        ub = f"b{b}"
                        # x1T = wo.T @ outn + bo + lqT   [D, L]
                        x1T = work.tile([P, DT, L], f32r, tag="x1T", bufs=1,
                                        name=f"x1T{ub}")
                        for mt in range(DT):
                            ps = psum.tile([P, L], f32, tag="work", bufs=3,
                                           name=f"psx1T{mt}{ub}")
                            for kt in range(DT):
                                nc.tensor.matmul(
                                    ps, wo_sb[:, kt, mt * P:(mt + 1) * P],
                                    outn[:, kt, :],
                                    start=(kt == 0), stop=(kt == DT - 1))
                            nc.vector.tensor_scalar(out=x1T[:, mt, :], in0=ps,
                                                    scalar1=bo_sb[:, mt:mt + 1],
                                                    scalar2=None, op0=ALU.add)
                            nc.vector.tensor_tensor(x1T[:, mt, :], x1T[:, mt, :],
                                                    lqT_sb[:, mt, :], ALU.add)

                        # x1n = outn.T @ wo + bo + lq   [L, D]
                        x1n = work.tile([P, LT, D], f32, tag="x1n", bufs=1,
                                        name=f"x1n{ub}")
                        for lt in range(LT):
                            ps = psum.tile([P, L], f32, tag="work", bufs=3,
                                           name=f"psx1n{lt}{ub}")
                            for kt in range(DT):
                                nc.tensor.matmul(
                                    ps[:, 0:D],
                                    outn[:, kt, lt * P:(lt + 1) * P],
                                    wo_sb[:, kt, :],
                                    start=(kt == 0), stop=(kt == DT - 1))
                            nc.vector.tensor_tensor(x1n[:, lt, :], ps[:, 0:D], bo_b,
                                                    ALU.add)
                            nc.vector.tensor_tensor(x1n[:, lt, :], x1n[:, lt, :],
                                                    lqn_sb[:, lt, :], ALU.add)

                        # ---- PreNorm GEGLU feedforward on x1T ----
                        ps_mb2, ps_ab2 = ln_stats(x1T, L, f"f{ub}")
                        fT = work.tile([P, DT, L], bf16, tag="fT", bufs=1,
                                       name=f"fT{ub}")
                        for kt in range(DT):
                            t = work.tile([P, L], f32, tag="sc", bufs=3,
                                          name=f"fTt{kt}{ub}")
                            nc.vector.tensor_tensor(t, x1T[:, kt, :], ps_mb2,
                                                    ALU.subtract)
                            nc.vector.tensor_tensor(fT[:, kt, :], t, ps_ab2,
                                                    ALU.mult)
                        return fT, x1n

                    # epilogue part 3: GEGLU FF + output
                    def epi3(fT, x1n):
                        ub = f"b{b}"
                        f2 = work.tile([P, FFT, L], bf16, tag="f2", bufs=1,
                                       name=f"f2{ub}")
                        for mt in range(FFT):
                            ps_a = psum.tile([P, L], f32, tag="work", bufs=3,
                                             name=f"psfa{mt}{ub}")
                            ps_g = psum.tile([P, L], f32, tag="work", bufs=3,
                                             name=f"psfg{mt}{ub}")
                            for kt in range(DT):
                                nc.tensor.matmul(
                                    ps_a, fw1_sb[:, kt, mt * P:(mt + 1) * P], fT[:, kt, :],
                                    start=(kt == 0), stop=(kt == DT - 1))
                            for kt in range(DT):
                                nc.tensor.matmul(
                                    ps_g, fw1_sb[:, kt, (FFT + mt) * P:(FFT + mt + 1) * P],
                                    fT[:, kt, :],
                                    start=(kt == 0), stop=(kt == DT - 1))
                            g_sb = work.tile([P, L], bf16, tag="g", bufs=2,
                                             name=f"g{mt}{ub}")
                            nc.scalar.activation(g_sb, ps_g, gelu_af,
                                                 bias=bu_sb[:, FFT + mt:FFT + mt + 1],
                                                 scale=1.0)
                            t2 = work.tile([P, L], f32, tag="sc", bufs=3,
                                           name=f"f2t{mt}{ub}")
                            nc.vector.tensor_scalar(out=t2, in0=ps_a,
                                                    scalar1=bu_sb[:, mt:mt + 1],
                                                    scalar2=None, op0=ALU.add)
                            nc.vector.tensor_tensor(f2[:, mt, :], t2, g_sb, ALU.mult)

                        # y = f2.T @ fw2 + fb2 + x1n  [L, D]
                        y_sb = work.tile([P, LT, D], f32, tag="y", bufs=1,
                                         name=f"y{ub}")
                        for lt in range(LT):
                            ps = psum.tile([P, L], f32, tag="work", bufs=3,
                                           name=f"psy{lt}{ub}")
                            for kt in range(FFT):
                                nc.tensor.matmul(
                                    ps[:, 0:D], f2[:, kt, lt * P:(lt + 1) * P],
                                    fw2_sb[:, kt, :],
                                    start=(kt == 0), stop=(kt == FFT - 1))
                            nc.vector.tensor_tensor(y_sb[:, lt, :], ps[:, 0:D], fb2_b,
                                                    ALU.add)
                            nc.vector.tensor_tensor(y_sb[:, lt, :], y_sb[:, lt, :],
                                                    x1n[:, lt, :], ALU.add)
                        nc.sync.dma_start(y[b].rearrange("(t p) d -> p t d", p=P),
                                          y_sb)

                    return stage_a, stage_b, epi1, epi2, epi3

                # orchestrate: 1-chunk skew within a batch; spread batch
                # b-1's epilogue parts across batch b's chunk stream
                pending = None       # (e2, outn, e3) from previous batch
                pending_res = None   # (fT, x1n) after e2 ran

                def flush_pending():
                    nonlocal pending, pending_res
                    if pending is not None:
                        if pending_res is None:
                            pending_res = pending[0](pending[1])
                        pending[2](*pending_res)
                        pending = None
                        pending_res = None

                for b in range(bpc):
                    sa, sb_, e1, e2, e3 = make_batch(b)
                    pend = None
                    for c in range(nchunks):
                        cn_c = sa(c)
                        if pending is not None:
                            if c == 1 and pending_res is None:
                                pending_res = pending[0](pending[1])
                            elif c == 3:
                                flush_pending()
                        if pend is not None:
                            sb_(*pend)
                        pend = (c, cn_c)
                    sb_(*pend)
                    flush_pending()
                    outn_b = e1()
                    pending = (e2, outn_b, e3)
                    pending_res = None
                flush_pending()

            if repeat is not None and repeat > 1:
                with tc.For_i(0, repeat, 1):
                    _run_batches()
            else:
                _run_batches()

    nc.compile()
    return nc


def host_prep(inputs, n_points=N_FULL):
    """Fold LN gains, precompute query path, build per-core input maps."""
    f = lambda a: np.ascontiguousarray(np.asarray(a), dtype=np.float32)
    x = f(inputs["x"])[:, :n_points, :]
    query = f(inputs["query"])[0]  # [L, D]

    # query path (batch-independent): q = LN(query) @ wq
    g, bb = f(inputs["ln_q_g"]), f(inputs["ln_q_b"])
    m = query.mean(-1, keepdims=True)
    v = query.var(-1, keepdims=True)
    qn = (query - m) / np.sqrt(v + EPS) * g + bb
    q = qn @ f(inputs["wq"])  # [L, D]

    wkv = f(inputs["wkv"]) * f(inputs["ln_ctx_g"])[:, None]
    bkv = f(inputs["ln_ctx_b"]) @ f(inputs["wkv"])
    fw1 = f(inputs["ff_w1"]) * f(inputs["ln_ff_g"])[:, None]
    bu = f(inputs["ff_b1"]) + f(inputs["ln_ff_b"]) @ f(inputs["ff_w1"])

    wq2 = np.ascontiguousarray(wkv[:, :D] @ q.T)  # [D, L]
    common = {
        "wq2": wq2,
        "lqT": np.ascontiguousarray(query.T),
        "lqn": query,
        "w0": f(inputs["mlp_w0"]), "b0": f(inputs["mlp_b0"]),
        "w1": f(inputs["mlp_w1"]), "b1": f(inputs["mlp_b1"]),
        "w2": f(inputs["mlp_w2"]), "b2": f(inputs["mlp_b2"]),
        "w3": f(inputs["mlp_w3"]), "b3": f(inputs["mlp_b3"]),
        "wkv": wkv, "bkv": bkv,
        "wo": f(inputs["wo"]), "bo": f(inputs["bo"]),
        "fw1": fw1.astype(ml_dtypes.bfloat16), "bu": bu,
        "fw2": f(inputs["ff_w2"]).astype(ml_dtypes.bfloat16),
        "fb2": f(inputs["ff_b2"]),
    }
    in_maps = []
    for c in range(NCORES):
        xs = x[c * BPC:(c + 1) * BPC]  # [BPC, n, 3]
        xTs = np.ascontiguousarray(xs.transpose(0, 2, 1))  # [BPC, 3, n]
        in_maps.append({"xT": xTs, **common})
    return in_maps


_NC_CACHE = {}


def get_nc(n_points=N_FULL):
    if n_points not in _NC_CACHE:
        _NC_CACHE[n_points] = build_nc(n_points)
    return _NC_CACHE[n_points]


def kernel(**inputs):
    from concourse.bass_utils import run_bass_kernel_spmd
    nc = get_nc(N_FULL)
    in_maps = host_prep(inputs, N_FULL)
    res = run_bass_kernel_spmd(nc, in_maps, core_ids=list(range(NCORES)))
    y = np.concatenate([r["y"] for r in res.results], axis=0)
    return y.astype(np.float32)

